# revision 19
# baseline (speedup 1.0000x reference)
"""DCGCN forward on 8 trn2 NeuronCores.

Sharding: data-parallel over batch B=8, one batch element per core.
Layout: everything feature-major ("_T"): tensors [feat, node] with feat on
partitions, so PE matmuls never need activations transposed, softmax runs
along the free dim, and LN scale/bias are per-partition ACT scalars.

Softmax: no max-subtraction (leaky_relu bounds scores below; constant bias
C=25 in the exp handles overflow; softmax is shift-invariant).

Precision: matmuls in fp32 (native 4 cyc/row) except the attention-weight
matmul c = Um @ x which runs f32r (Um single-rounded, x split hi+lo).
"""
import numpy as np
from contextlib import ExitStack

import concourse.bass as bass
import concourse.mybir as mybir
import concourse.tile as tile
from concourse import bacc
from concourse.bass_utils import run_bass_kernel_spmd
from concourse.masks import make_identity

F32 = mybir.dt.float32
F32R = mybir.dt.float32r
BF16 = mybir.dt.bfloat16
I32 = mybir.dt.int32
AF = mybir.ActivationFunctionType
OP = mybir.AluOpType

N = 1024
D = 768
HALF = 384
LEAK = 0.01
CEXP = 25.0
LN_EPS = 1e-5
LAYERS = [(6, 64), (3, 128), (6, 64), (3, 128)]
CELLS = ("d", "r")


def _mm(nc, out, lhsT, rhs, start, stop, fmax=512, skip=False):
    """matmul accumulating into psum `out`; splits moving dim into <=fmax."""
    n = rhs.shape[-1]
    o = 0
    while o < n:
        w = min(fmax, n - o)
        nc.tensor.matmul(out[:, o:o + w], lhsT, rhs[:, o:o + w],
                         start=start, stop=stop, skip_group_check=skip)
        o += w


def build_program(adj_words):
    """adj_words: 1 if adj arrives int32, 2 if int64 (viewed as int32 pairs)."""
    nc = bacc.Bacc(None, target_bir_lowering=False)

    adj_in = nc.declare_dram_parameter("adj", [N, N * adj_words], I32, isOutput=False)
    h_in = nc.declare_dram_parameter("h", [N, D], F32, isOutput=False)
    out_dram = nc.declare_dram_parameter("out", [N, D], F32, isOutput=True)

    # params
    pw = {}
    for c in CELLS:
        pw[f"{c}_in_w"] = nc.declare_dram_parameter(f"{c}_in_w", [D, HALF], F32, isOutput=False)
        pw[f"{c}_in_b"] = nc.declare_dram_parameter(f"{c}_in_b", [HALF, 1], F32, isOutput=False)
        for l, (heads, hid) in enumerate(LAYERS):
            for i in range(heads):
                kh = HALF + hid * i
                pw[f"{c}_L{l}_fc{i}_w"] = nc.declare_dram_parameter(f"{c}_L{l}_fc{i}_w", [kh, hid], F32, isOutput=False)
                pw[f"{c}_L{l}_fc{i}_b"] = nc.declare_dram_parameter(f"{c}_L{l}_fc{i}_b", [hid, 1], F32, isOutput=False)
                pw[f"{c}_L{l}_at{i}"] = nc.declare_dram_parameter(f"{c}_L{l}_at{i}", [hid, hid], F32, isOutput=False)
            pw[f"{c}_L{l}_lw"] = nc.declare_dram_parameter(f"{c}_L{l}_lw", [HALF, HALF], F32, isOutput=False)
            pw[f"{c}_L{l}_lb"] = nc.declare_dram_parameter(f"{c}_L{l}_lb", [HALF, 1], F32, isOutput=False)
            pw[f"{c}_L{l}_g"] = nc.declare_dram_parameter(f"{c}_L{l}_g", [HALF, 1], F32, isOutput=False)
            pw[f"{c}_L{l}_be"] = nc.declare_dram_parameter(f"{c}_L{l}_be", [HALF, 1], F32, isOutput=False)

    with tile.TileContext(nc) as tc, ExitStack() as ctx:
        const = ctx.enter_context(tc.tile_pool(name="const", bufs=1))
        slabp = ctx.enter_context(tc.tile_pool(name="slab", bufs=1))
        psx = ctx.enter_context(tc.tile_pool(name="psx", bufs=1, space="PSUM"))
        ident = const.tile([128, 128], F32)
        make_identity(nc, ident)

        def col(val, tag):
            t = const.tile([128, 1], F32, tag=tag)
            nc.vector.memset(t, val)
            return t
        eps_col = col(LN_EPS, "eps")
        negc_col = col(-CEXP, "negc")
        ones_col = col(1.0, "ones")

        # ---- masks: build in natural layout, bf16-transpose to _T ----
        masks = {}
        for cname in CELLS:
            mT = slabp.tile([128, 8, N], BF16, tag=f"mask_{cname}")
            masks[cname] = mT
        for half in range(2):
            with tc.tile_pool(name="adjp", bufs=1) as adjp:
                adj_sb = adjp.tile([128, 4, N * adj_words], I32, tag="adj_sb")
                nc.sync.dma_start(
                    out=adj_sb,
                    in_=adj_in.rearrange("(t p) w -> p t w", p=128)[:, 4 * half:4 * half + 4, :])
                if adj_words == 2:
                    av = adj_sb.rearrange("p t (m two) -> p t m two", two=2)[:, :, :, 0]
                else:
                    av = adj_sb[:, :, :]
                af = adjp.tile([128, 4, N], F32, tag="af")
                nc.vector.tensor_copy(out=af, in_=av)
                eq1 = adjp.tile([128, 4, N], BF16, tag="eq1")
                nc.vector.tensor_scalar(out=eq1, in0=af, scalar1=1.0, scalar2=None,
                                        op0=OP.is_equal)
                dirn = adjp.tile([128, 4, N], BF16, tag="dirn")
                nc.vector.tensor_scalar(out=dirn, in0=af, scalar1=2.5, scalar2=None,
                                        op0=OP.is_ge)
                nc.vector.tensor_tensor(out=dirn, in0=dirn, in1=eq1, op=OP.add)
                revn = adjp.tile([128, 4, N], BF16, tag="revn")
                nc.vector.tensor_scalar(out=revn, in0=af, scalar1=1.5, scalar2=None,
                                        op0=OP.is_ge)
                for cname, mnat in (("d", dirn), ("r", revn)):
                    for t in range(4):
                        tt = 4 * half + t
                        nc.sync.dma_start_transpose(
                            masks[cname][:, :, 128 * tt:128 * (tt + 1)], mnat[:, t, :])

        # ---- h transpose: h [N, D] -> hT [128, 6, N] fp32 ----
        slabs = {}
        with tc.tile_pool(name="hp", bufs=1) as hp, \
             tc.tile_pool(name="inwp", bufs=1) as inwp:
            h_nat = hp.tile([128, 8, D], F32)
            nc.sync.dma_start(out=h_nat, in_=h_in.rearrange("(t p) d -> p t d", p=128))
            hT = hp.tile([128, 6, N], F32)
            for j in range(6):
                ph = psx.tile([128, N], F32, tag="ps_big")
                for t in range(8):
                    nc.tensor.transpose(ph[:, 128 * t:128 * (t + 1)],
                                        h_nat[:, t, 128 * j:128 * (j + 1)], ident)
                nc.vector.tensor_copy(out=hT[:, j, :], in_=ph)

            # input projection per cell: slab rows 0:384 = hT @ in_w + b
            for c in CELLS:
                slab = slabp.tile([128, 6, N], F32, tag=f"slab_{c}")
                slabs[c] = slab
                w_sb = inwp.tile([128, 6, HALF], F32, tag="in_w")
                nc.sync.dma_start(out=w_sb, in_=pw[f"{c}_in_w"].rearrange("(j p) m -> p j m", p=128))
                b_sb = inwp.tile([128, 3], F32, tag="in_b")
                nc.sync.dma_start(out=b_sb, in_=pw[f"{c}_in_b"].rearrange("(g p) o -> p (g o)", p=128))
                for g in range(3):
                    px = psx.tile([128, N], F32, tag="ps_big")
                    for j in range(6):
                        _mm(nc, px, w_sb[:, j, 128 * g:128 * (g + 1)], hT[:, j, :],
                            start=(j == 0), stop=(j == 5))
                    nc.scalar.activation(out=slab[:, g, :], in_=px, func=AF.Identity,
                                         bias=b_sb[:, g:g + 1], scale=1.0)

        wpool = ctx.enter_context(tc.tile_pool(name="wts", bufs=2))
        work = ctx.enter_context(tc.tile_pool(name="work", bufs=2))
        work1 = ctx.enter_context(tc.tile_pool(name="work1", bufs=1))
        small = ctx.enter_context(tc.tile_pool(name="small", bufs=4))
        pss = ctx.enter_context(tc.tile_pool(name="pss", bufs=2, space="PSUM"))
        psc = ctx.enter_context(tc.tile_pool(name="psc", bufs=1, space="PSUM"))
        dramp = ctx.enter_context(tc.tile_pool(name="dram", bufs=4, space="DRAM"))

        # ---- per-layer / per-head ----
        def head(c, l, i, heads, hid):
            slab = slabs[c]
            kh = HALF + hid * i
            nchunk = (kh + 127) // 128
            # weights
            wfc = wpool.tile([128, nchunk, hid], F32, tag="fc_w")
            kfull = kh - (nchunk - 1) * 128
            src = pw[f"{c}_L{l}_fc{i}_w"]
            for j in range(nchunk):
                k0 = 128 * j
                kk = min(128, kh - k0)
                nc.sync.dma_start(out=wfc[0:kk, j, :], in_=src[k0:k0 + kk, :])
            bfc = small.tile([hid, 1], F32, tag="fc_b")
            nc.sync.dma_start(out=bfc, in_=pw[f"{c}_L{l}_fc{i}_b"][:, :])
            wat = wpool.tile([hid, hid], F32, tag="at_w")
            nc.sync.dma_start(out=wat, in_=pw[f"{c}_L{l}_at{i}"][:, :])

            # x_T = slab[:kh] @ wfc + b   [hid, N]
            px = psx.tile([hid, N], F32, tag="ps_big")
            for j in range(nchunk):
                kk = min(128, kh - 128 * j)
                _mm(nc, px, wfc[0:kk, j, :], slab[0:kk, j, :],
                    start=(j == 0), stop=(j == nchunk - 1))
            x_sb = work.tile([hid, N], F32, tag="x_sb")
            nc.scalar.activation(out=x_sb, in_=px, func=AF.Identity,
                                 bias=bfc[:, 0:1], scale=1.0)

            # y_T = x_T @ wat  [hid, N]
            py = psx.tile([hid, N], F32, tag="ps_big")
            _mm(nc, py, wat, x_sb, start=True, stop=True)
            y_sb = work.tile([hid, N], F32, tag="y_sb")
            nc.scalar.activation(out=y_sb, in_=py, func=AF.Identity, bias=0.0, scale=1.0)

            # x natural (hi/lo f32r) via PE transpose.
            # xnh layout: [x[:,0:64] | ones | x[:,64:hid]]  (width hid+1)
            pxn = psx.tile([128, 8 * hid], F32, tag="ps_big")
            for t in range(8):
                nc.tensor.transpose(pxn[:, hid * t:hid * (t + 1)],
                                    x_sb[:, 128 * t:128 * (t + 1)],
                                    ident[0:hid, 0:hid])
            xnh = work.tile([128, 8, hid + 1], F32R, tag="xnh")
            pxn3 = pxn.rearrange("p (t q) -> p t q", q=hid)
            nc.vector.memset(xnh[:, :, :].bitcast(F32), 1.0)
            nc.vector.tensor_copy(out=xnh[:, :, 0:64], in_=pxn3[:, :, 0:64])
            if hid == 128:
                nc.vector.tensor_copy(out=xnh[:, :, 65:hid + 1], in_=pxn3[:, :, 64:hid])
            xnl = work.tile([128, 8, hid], F32R, tag="xnl")
            xnl_hi_view = None  # hi parts matching xnl col order
            nc.vector.scalar_tensor_tensor(out=xnl[:, :, 0:64], in0=pxn3[:, :, 0:64],
                                           scalar=1.0, in1=xnh[:, :, 0:64],
                                           op0=OP.mult, op1=OP.subtract)
            if hid == 128:
                nc.vector.scalar_tensor_tensor(out=xnl[:, :, 64:128], in0=pxn3[:, :, 64:128],
                                               scalar=1.0, in1=xnh[:, :, 65:129],
                                               op0=OP.mult, op1=OP.subtract)

            # groups: (hi-slice in xnh, lo-slice in xnl, #real cols, has ones)
            if hid == 64:
                groups = [((0, 65), (0, 64), 64, True)]
            else:
                groups = [((0, 65), (0, 64), 64, True), ((65, 129), (64, 128), 64, False)]
            pcs = []
            for gi, (hslc, lslc, wcols, withones) in enumerate(groups):
                pc_g = psc.tile([wcols + (1 if withones else 0), N], F32,
                                tag=f"ps_c{gi}")
                pcs.append(pc_g)

            # S_T per m-tile -> prelu -> exp -> mask -> Um (streamed) -> c accum
            mT = masks[c]
            for mt in range(8):
                um = work.tile([128, N], F32R, tag="um")
                for hh in range(2):
                    ps = pss.tile([128, 512], F32, tag="ps_s")
                    nc.tensor.matmul(ps, x_sb[:, 128 * mt:128 * (mt + 1)],
                                     y_sb[:, 512 * hh:512 * (hh + 1)],
                                     start=True, stop=True)
                    lk = work.tile([128, 512], F32, tag="lk")
                    nc.scalar.activation(out=lk, in_=ps, func=AF.Prelu,
                                         bias=0.0, scale=1.0, alpha=LEAK)
                    ex = work.tile([128, 512], F32, tag="ex")
                    nc.scalar.activation(out=ex, in_=lk, func=AF.Exp,
                                         bias=negc_col[:, 0:1], scale=1.0)
                    nc.vector.tensor_tensor(
                        out=um[:, 512 * hh:512 * (hh + 1)], in0=ex,
                        in1=mT[:, mt, 512 * hh:512 * (hh + 1)], op=OP.mult)
                for gi, (hslc, lslc, wcols, withones) in enumerate(groups):
                    pc = pcs[gi]
                    _mm(nc, pc, xnh[:, mt, hslc[0]:hslc[1]], um,
                        start=(mt == 0), stop=False, skip=True)
                    _mm(nc, pc[0:wcols, :], xnl[:, mt, lslc[0]:lslc[1]], um,
                        start=False, stop=(mt == 7), skip=True)

            # recip of rowsums r = pcs[0][wcols] via DRAM bounce
            rec_bc = work1.tile([128, N], F32, tag="rec_bc")
            rrow = work1.tile([1, N], F32, tag="row")
            nc.scalar.copy(out=rrow, in_=pcs[0][64:65, :])
            da = dramp.tile([1, N], F32, tag="da")
            nc.sync.dma_start(out=da, in_=rrow)
            r8 = small.tile([128, 8], F32, tag="r8")
            nc.sync.dma_start(out=r8, in_=da.rearrange("o (t p) -> (o p) t", p=128))
            rec8 = small.tile([128, 8], F32, tag="rec8")
            nc.vector.reciprocal(rec8, r8)
            db = dramp.tile([1, N], F32, tag="db")
            nc.sync.dma_start(out=db.rearrange("o (t p) -> (o p) t", p=128), in_=rec8)
            nc.sync.dma_start(out=rec_bc, in_=db.to_broadcast((128, N)))

            # evict: slab rows = relu(c) * rec_bc
            for gi, (hslc, lslc, wcols, withones) in enumerate(groups):
                rowbase = HALF + hid * i + 64 * gi
                p0, j0 = rowbase % 128, rowbase // 128
                nc.vector.scalar_tensor_tensor(
                    out=slabs[c][p0:p0 + wcols, j0, :], in0=pcs[gi][0:wcols, :],
                    scalar=0.0, in1=rec_bc[p0:p0 + wcols, :],
                    op0=OP.max, op1=OP.mult)

        def tail(c, l):
            slab = slabs[c]
            wl = wpool.tile([128, 3, HALF], F32, tag="lw")
            nc.sync.dma_start(out=wl, in_=pw[f"{c}_L{l}_lw"].rearrange("(j p) m -> p j m", p=128))
            lb = small.tile([128, 3], F32, tag="lb")
            nc.sync.dma_start(out=lb, in_=pw[f"{c}_L{l}_lb"].rearrange("(g p) o -> p (g o)", p=128))
            gg = small.tile([128, 3], F32, tag="gg")
            nc.sync.dma_start(out=gg, in_=pw[f"{c}_L{l}_g"].rearrange("(g p) o -> p (g o)", p=128))
            bb = small.tile([128, 3], F32, tag="bb")
            nc.sync.dma_start(out=bb, in_=pw[f"{c}_L{l}_be"].rearrange("(g p) o -> p (g o)", p=128))

            o3s = []
            pm = psc.tile([1, N], F32, tag="ps_c0")
            for g in range(3):
                pr = psx.tile([128, N], F32, tag="ps_big")
                for j in range(3):
                    _mm(nc, pr, wl[:, j, 128 * g:128 * (g + 1)], slab[:, 3 + j, :],
                        start=(j == 0), stop=(j == 2))
                rr = work1.tile([128, N], F32, tag="rr")
                nc.scalar.activation(out=rr, in_=pr, func=AF.Relu,
                                     bias=lb[:, g:g + 1], scale=1.0)
                o3 = work1.tile([128, N], F32, tag=f"o3_{g}")
                nc.vector.tensor_tensor(out=o3, in0=rr, in1=slab[:, g, :], op=OP.add)
                o3s.append(o3)
                _mm(nc, pm, ones_col[:, 0:1], o3, start=(g == 0), stop=(g == 2))
            psq = psc.tile([1, N], F32, tag="ps_c1")
            for g in range(3):
                sq = work1.tile([128, N], F32, tag="rr")
                nc.scalar.activation(out=sq, in_=o3s[g], func=AF.Square, bias=0.0, scale=1.0)
                _mm(nc, psq, ones_col[:, 0:1], sq, start=(g == 0), stop=(g == 2))

            mrow = work1.tile([1, N], F32, tag="row")
            nc.scalar.copy(out=mrow, in_=pm)
            qrow = work1.tile([1, N], F32, tag="row")
            nc.scalar.copy(out=qrow, in_=psq)
            dm = dramp.tile([1, N], F32, tag="dm")
            nc.sync.dma_start(out=dm, in_=mrow)
            dq = dramp.tile([1, N], F32, tag="dq")
            nc.sync.dma_start(out=dq, in_=qrow)
            m8 = small.tile([128, 8], F32, tag="m8")
            nc.sync.dma_start(out=m8, in_=dm.rearrange("o (t p) -> (o p) t", p=128))
            q8 = small.tile([128, 8], F32, tag="q8")
            nc.sync.dma_start(out=q8, in_=dq.rearrange("o (t p) -> (o p) t", p=128))
            mu8 = small.tile([128, 8], F32, tag="mu8")
            nc.vector.tensor_scalar(out=mu8, in0=m8, scalar1=1.0 / HALF, scalar2=None, op0=OP.mult)
            mu2 = small.tile([128, 8], F32, tag="mu2")
            nc.vector.tensor_tensor(out=mu2, in0=mu8, in1=mu8, op=OP.mult)
            var8 = small.tile([128, 8], F32, tag="var8")
            # var = q/384 - mu^2
            nc.vector.scalar_tensor_tensor(out=var8, in0=q8, scalar=1.0 / HALF,
                                           in1=mu2, op0=OP.mult, op1=OP.subtract)
            lv = small.tile([128, 8], F32, tag="lv")
            nc.scalar.activation(out=lv, in_=var8, func=AF.Ln, bias=eps_col[:, 0:1], scale=1.0)
            rstd8 = small.tile([128, 8], F32, tag="rstd8")
            nc.scalar.activation(out=rstd8, in_=lv, func=AF.Exp, bias=0.0, scale=-0.5)
            nm8 = small.tile([128, 8], F32, tag="nm8")
            nc.vector.scalar_tensor_tensor(out=nm8, in0=mu8, scalar=-1.0,
                                           in1=rstd8, op0=OP.mult, op1=OP.mult)
            dr = dramp.tile([1, N], F32, tag="dr")
            nc.sync.dma_start(out=dr.rearrange("o (t p) -> (o p) t", p=128), in_=rstd8)
            dn = dramp.tile([1, N], F32, tag="dn")
            nc.sync.dma_start(out=dn.rearrange("o (t p) -> (o p) t", p=128), in_=nm8)
            rstd_bc = work1.tile([128, N], F32, tag="rec_bc")
            nc.sync.dma_start(out=rstd_bc, in_=dr.to_broadcast((128, N)))
            nm_bc = work1.tile([128, N], F32, tag="nm_bc")
            nc.sync.dma_start(out=nm_bc, in_=dn.to_broadcast((128, N)))

            for g in range(3):
                o3 = o3s[g]
                nc.vector.tensor_tensor(out=o3, in0=o3, in1=rstd_bc, op=OP.mult)
                nc.vector.tensor_tensor(out=o3, in0=o3, in1=nm_bc, op=OP.add)
                nc.vector.tensor_scalar(out=slab[:, g, :], in0=o3,
                                        scalar1=gg[:, g:g + 1],
                                        scalar2=bb[:, g:g + 1],
                                        op0=OP.mult, op1=OP.add)

        for l, (heads, hid) in enumerate(LAYERS):
            for i in range(heads):
                for c in CELLS:
                    head(c, l, i, heads, hid)
            for c in CELLS:
                tail(c, l)

        # ---- output: transpose slab rows 0:384 of each cell -> out ----
        for ci, c in enumerate(CELLS):
            slab = slabs[c]
            for t in range(8):
                po = psx.tile([128, HALF], F32, tag="ps_big")
                for g in range(3):
                    nc.tensor.transpose(po[:, 128 * g:128 * (g + 1)],
                                        slab[:, g, 128 * t:128 * (t + 1)], ident)
                osb = work1.tile([128, HALF], F32, tag="rr")
                nc.vector.tensor_copy(out=osb, in_=po)
                nc.sync.dma_start(
                    out=out_dram[128 * t:128 * (t + 1), HALF * ci:HALF * (ci + 1)],
                    in_=osb)

    nc.finalize()
    return nc


_CACHE = {}


def _flat_params(params):
    out = {}
    for c, key in (("d", "directed"), ("r", "reversed")):
        cp = params[key]
        out[f"{c}_in_w"] = np.ascontiguousarray(np.asarray(cp["in_w"], np.float32))
        out[f"{c}_in_b"] = np.asarray(cp["in_b"], np.float32).reshape(-1, 1)
        for l, lp in enumerate(cp["layers"]):
            for i in range(len(lp["fc_w"])):
                out[f"{c}_L{l}_fc{i}_w"] = np.ascontiguousarray(np.asarray(lp["fc_w"][i], np.float32))
                out[f"{c}_L{l}_fc{i}_b"] = np.asarray(lp["fc_b"][i], np.float32).reshape(-1, 1)
                out[f"{c}_L{l}_at{i}"] = np.ascontiguousarray(np.asarray(lp["attn_W"][i], np.float32))
            out[f"{c}_L{l}_lw"] = np.ascontiguousarray(np.asarray(lp["layer_fc_w"], np.float32))
            out[f"{c}_L{l}_lb"] = np.asarray(lp["layer_fc_b"], np.float32).reshape(-1, 1)
            out[f"{c}_L{l}_g"] = np.asarray(lp["ln_g"], np.float32).reshape(-1, 1)
            out[f"{c}_L{l}_be"] = np.asarray(lp["ln_b"], np.float32).reshape(-1, 1)
    return out


def kernel(adj, h, params, _trace=False):
    adj = np.asarray(adj)
    h = np.asarray(h, np.float32)
    B = adj.shape[0]
    if adj.dtype == np.int64:
        adj32 = adj.view(np.int32).reshape(B, N, 2 * N)
        words = 2
    else:
        adj32 = np.ascontiguousarray(adj.astype(np.int32, copy=False)).reshape(B, N, N)
        words = 1
    if words not in _CACHE:
        _CACHE[words] = build_program(words)
    nc = _CACHE[words]
    fp = _flat_params(params)
    core_ids = list(range(8))
    in_maps = []
    for b in range(8):
        m = {"adj": adj32[b], "h": np.ascontiguousarray(h[b])}
        m.update(fp)
        in_maps.append(m)
    res = run_bass_kernel_spmd(nc, in_maps, core_ids)
    out = np.stack([res.results[b]["out"] for b in range(8)], axis=0)
    if _trace:
        return out.astype(np.float32), res
    return out.astype(np.float32)
